# revision 22
# baseline (speedup 1.0000x reference)
"""Decomposed multi-head attention for trn2, 8 NeuronCores.

Sharding: core c = b*4 + g handles batch b (of 2) and head-group g (4 heads,
256 dims of E). Each core computes its heads' q/k/v projections, attention
(scores -> exp -> attn@v with a ones-column appended to V for the softmax
denominator), and a partial out-projection over its 256-dim column block of
Wo. The four partials per batch are summed on the host; bq/bk are added on
device, bv/bo fold into a constant row added on the host (bv @ Wo.T + bo).

All device matmuls run as out = lhsT.T @ rhs with operands packed on the
host into the exact SBUF layouts (so every DMA is contiguous per partition)
and typed float32r for single-pass PE throughput. Attention is software-
pipelined: attn@v trails scores/exp by one k-tile so the PE never stalls on
ScalarE and the HAM clock gate stays open.
"""

import numpy as np
from contextlib import ExitStack

import concourse.bass as bass
import concourse.bacc as bacc
import concourse.mybir as mybir
import concourse.tile as tile
from concourse.bass_utils import run_bass_kernel_spmd

B, S, E, H, D = 2, 2048, 1024, 16, 64
P = 128
NCORES = 8
GROUPS = 4                # head-groups across cores (x batch = 8 cores)
HPC = H // GROUPS         # 4 heads per core
EG = HPC * D              # 256 dims of E per core
KE = E // P               # 8 k-tiles for the projections
NB = 4                    # 512-wide S blocks in the projection phase
SW = S // NB              # 512
ST = S // P               # 16 S-tiles for v
KT = S // P               # 16 k-tiles in attention
QW = 1024                 # q-block width in attention
QB = S // QW              # 2 q-blocks
F32 = mybir.dt.float32
F32R = mybir.dt.float32r

_NC = None
LAST_RESULT = None        # BassKernelResults of the most recent run (for test.py)


def _build_program():
    nc = bacc.Bacc("TRN2", target_bir_lowering=False, debug=False,
                   num_devices=NCORES)
    AF = mybir.ActivationFunctionType

    # Host-packed layouts: every DMA below is contiguous per partition.
    xq = nc.declare_dram_parameter("xq", [NB, P, KE, SW], F32R, isOutput=False)[:]
    xk = nc.declare_dram_parameter("xk", [NB, P, KE, SW], F32R, isOutput=False)[:]
    xv = nc.declare_dram_parameter("xv", [NB, P, KE, SW], F32R, isOutput=False)[:]
    wq = nc.declare_dram_parameter("wq", [P, KE, EG], F32R, isOutput=False)[:]
    wk = nc.declare_dram_parameter("wk", [P, KE, EG], F32R, isOutput=False)[:]
    wv = nc.declare_dram_parameter("wv", [P, KE, EG], F32R, isOutput=False)[:]
    wo = nc.declare_dram_parameter("wo", [D, HPC, E], F32R, isOutput=False)[:]
    bqk = nc.declare_dram_parameter("bqk", [P, 4], F32, isOutput=False)[:]
    vones = nc.declare_dram_parameter("vones", [P, HPC], F32R, isOutput=False)[:]
    outT = nc.declare_dram_parameter("outT", [E, S], F32, isOutput=True)[:]

    out_t = outT.rearrange("(m p) s -> p m s", p=P)

    with tile.TileContext(nc) as tc, ExitStack() as ctx:
        wpool = ctx.enter_context(tc.tile_pool(name="w", bufs=1))
        xpool = ctx.enter_context(tc.tile_pool(name="x", bufs=2))
        qkpool = ctx.enter_context(tc.tile_pool(name="qk", bufs=1))
        vpool = ctx.enter_context(tc.tile_pool(name="v", bufs=1))
        expool = ctx.enter_context(tc.tile_pool(name="ex", bufs=6))
        ohpool = ctx.enter_context(tc.tile_pool(name="oh", bufs=1))
        npool = ctx.enter_context(tc.tile_pool(name="n", bufs=2))
        outpool = ctx.enter_context(tc.tile_pool(name="ot", bufs=2))
        psum = ctx.enter_context(tc.tile_pool(name="ps", bufs=2, space="PSUM"))
        opsum = ctx.enter_context(tc.tile_pool(name="os", bufs=2, space="PSUM"))

        # --- resident weights ---
        wq_sb = wpool.tile([P, KE, EG], F32R, tag="wq")
        nc.sync.dma_start(wq_sb[:], wq)
        wk_sb = wpool.tile([P, KE, EG], F32R, tag="wk")
        nc.sync.dma_start(wk_sb[:], wk)
        wv_sb = wpool.tile([P, KE, EG], F32R, tag="wv")
        nc.sync.dma_start(wv_sb[:], wv)
        # out-proj weights laid out per head-sized k-tile of 64 partitions
        wo_sb = wpool.tile([D, HPC, E], F32R, tag="wo")
        nc.sync.dma_start(wo_sb[:], wo)
        bqk_sb = wpool.tile([P, 4], F32, tag="bqk")
        nc.sync.dma_start(bqk_sb[:], bqk)

        # --- phase 1: q/k/v projections, streamed in 512-wide S blocks ---
        qTt = [[None] * NB for _ in range(2)]   # [m][nb] -> [P, 512] (dims, S)
        kTt = [[None] * NB for _ in range(2)]
        v_sb = [None] * ST                      # [P, HPC, D+1] per S-tile
        for nb in range(NB):
            xq_blk = xpool.tile([P, KE, SW], F32R, tag="xblk",
                                name=f"xq_blk{nb}")
            nc.sync.dma_start(xq_blk[:], xq[nb])
            for m in range(2):
                ps = psum.tile([P, SW], F32, tag="sc", name=f"psq{nb}{m}")
                for k in range(KE):
                    nc.tensor.matmul(
                        ps[:], wq_sb[:, k, m * P:(m + 1) * P],
                        xq_blk[:, k, :],
                        start=(k == 0), stop=(k == KE - 1))
                t = qkpool.tile([P, SW], F32R, tag=f"qT{m}{nb}",
                                name=f"qT{m}{nb}")
                qTt[m][nb] = t
                nc.vector.tensor_scalar_add(t[:], ps[:], bqk_sb[:, m:m + 1])
            xk_blk = xpool.tile([P, KE, SW], F32R, tag="xblk",
                                name=f"xk_blk{nb}")
            nc.sync.dma_start(xk_blk[:], xk[nb])
            for m in range(2):
                ps = psum.tile([P, SW], F32, tag="sc", name=f"psk{nb}{m}")
                for k in range(KE):
                    nc.tensor.matmul(
                        ps[:], wk_sb[:, k, m * P:(m + 1) * P],
                        xk_blk[:, k, :],
                        start=(k == 0), stop=(k == KE - 1))
                t = qkpool.tile([P, SW], F32R, tag=f"kT{m}{nb}",
                                name=f"kT{m}{nb}")
                kTt[m][nb] = t
                nc.vector.tensor_scalar_add(t[:], ps[:], bqk_sb[:, 2 + m:3 + m])
            xv_blk = xpool.tile([P, KE, SW], F32R, tag="xblk",
                                name=f"xv_blk{nb}")
            nc.sync.dma_start(xv_blk[:], xv[nb])
            for st4 in range(SW // P):
                st = nb * (SW // P) + st4
                ps = psum.tile([P, EG], F32, tag="sc", name=f"psv{st}")
                for k in range(KE):
                    nc.tensor.matmul(
                        ps[:], xv_blk[:, k, st4 * P:(st4 + 1) * P],
                        wv_sb[:, k, :],
                        start=(k == 0), stop=(k == KE - 1))
                vt = vpool.tile([P, HPC, D + 1], F32R, tag=f"v{st}",
                                name=f"v{st}")
                v_sb[st] = vt
                nc.sync.dma_start(vt[:, :, D:D + 1], vones.unsqueeze(2))
                nc.vector.tensor_copy(
                    vt[:, :, 0:D], ps[:].rearrange("p (h d) -> p h d", h=HPC))

        # --- phase 2: attention + out-projection per q-block ---
        # Heads run in pairs (2p, 2p+1): partition offsets 0/64 of the same
        # m-half, so their K=64 score matmuls use different PE row groups
        # and overlap. attn@v trails by one k-tile (software pipeline), so
        # every PE instruction's inputs are ready when it issues and the PE
        # stays continuously busy (HAM stays at K=8/8).
        for qb in range(QB):
            oh = [None] * HPC                   # [64, QW] per head
            for pr in range(HPC // 2):
                m = pr
                ps_o = [opsum.tile([D + 1, QW], F32, tag="oacc",
                                   name=f"ps_o{qb}{pr}{i}")
                        for i in range(2)]
                exs = {}                       # k -> [exA, exB]

                def attnv(kv):
                    for hh in range(2):
                        h = 2 * pr + hh
                        for j in range(2):
                            nc.tensor.matmul(
                                ps_o[hh][:, j * SW:(j + 1) * SW],
                                v_sb[kv][:, h, :],
                                exs[kv][hh][:, j * SW:(j + 1) * SW],
                                start=(kv == 0), stop=(kv == KT - 1))
                    del exs[kv]

                for k in range(KT):
                    ps_s = [psum.tile([P, QW], F32, tag="sc",
                                      name=f"ps_s{qb}{pr}{k}{i}")
                            for i in range(2)]
                    for hh in range(2):
                        po = hh * D
                        for j in range(2):
                            nc.tensor.matmul(
                                ps_s[hh][:, j * SW:(j + 1) * SW],
                                kTt[m][k // 4][po:po + D,
                                               (k % 4) * P:(k % 4 + 1) * P],
                                qTt[m][qb * 2 + j][po:po + D, :],
                                start=True, stop=True)
                    exs[k] = []
                    for hh in range(2):
                        ex = expool.tile([P, QW], F32R, tag="ex",
                                         name=f"ex{qb}{pr}{k}{hh}")
                        nc.scalar.activation(ex[:], ps_s[hh][:], AF.Exp,
                                             scale=float(1.0 / np.sqrt(D)))
                        exs[k].append(ex)
                    # attn@v trails by 2 k-tiles and is emitted in pairs so
                    # the PE sees ~4.4us dense stretches of full-K matmuls
                    # (lets the HAM activity window re-open the clock gate).
                    if k >= 3 and k % 2 == 1:
                        attnv(k - 3)
                        attnv(k - 2)
                for kv in (KT - 2, KT - 1):
                    attnv(kv)
                for hh in range(2):
                    h = 2 * pr + hh
                    rec = npool.tile([1, QW], F32, tag="rec",
                                     name=f"rec{qb}{h}")
                    nc.vector.reciprocal(rec[:], ps_o[hh][D:D + 1, :])
                    rb = npool.tile([D, QW], F32, tag="rb", name=f"rb{qb}{h}")
                    nc.gpsimd.partition_broadcast(rb[:], rec[:])
                    oht = ohpool.tile([D, QW], F32R, tag=f"oh{h}{qb}",
                                      name=f"oh{h}{qb}")
                    oh[h] = oht
                    nc.vector.tensor_mul(oht[:], ps_o[hh][0:D, :], rb[:])
            for mo in range(E // P):
                for nbq in range(QW // SW):
                    pso = psum.tile([P, SW], F32, tag="sc",
                                    name=f"pso{qb}{mo}{nbq}")
                    for kt in range(HPC):
                        nc.tensor.matmul(
                            pso[:], wo_sb[:, kt, mo * P:(mo + 1) * P],
                            oh[kt][:, nbq * SW:(nbq + 1) * SW],
                            start=(kt == 0), stop=(kt == HPC - 1))
                    ot = outpool.tile([P, SW], F32, tag="ot",
                                      name=f"ot{qb}{mo}{nbq}")
                    nc.vector.tensor_copy(ot[:], pso[:])
                    nc.sync.dma_start(
                        out_t[:, mo, qb * QW + nbq * SW: qb * QW + (nbq + 1) * SW],
                        ot[:])
    nc.compile()
    return nc


def _get_nc():
    global _NC
    if _NC is None:
        _NC = _build_program()
    return _NC


def _pack_x(xT):
    """[E, S] f32 -> [NB, P, KE, SW] so each DMA block is contiguous 16KB
    per partition: out[nb, p, k, s] = xT[k*P + p, nb*SW + s]."""
    return np.ascontiguousarray(
        xT.reshape(KE, P, NB, SW).transpose(2, 1, 0, 3))


def _pack_w(wT):
    """[E, EG] f32 -> [P, KE, EG]: out[p, k, m] = wT[k*P + p, m]."""
    return np.ascontiguousarray(wT.reshape(KE, P, EG).transpose(1, 0, 2))


def kernel(query, key_, value, Wq, bq, Wk, bk, Wv, bv, Wo, bo):
    global LAST_RESULT
    query = np.asarray(query, np.float32)
    key_ = np.asarray(key_, np.float32)
    value = np.asarray(value, np.float32)
    Wq, Wk, Wv, Wo = (np.asarray(w, np.float32) for w in (Wq, Wk, Wv, Wo))
    bq, bk, bv, bo = (np.asarray(b_, np.float32) for b_ in (bq, bk, bv, bo))

    in_maps = []
    for c in range(NCORES):
        b, g = divmod(c, GROUPS)
        rows = slice(g * EG, (g + 1) * EG)
        in_maps.append({
            "xq": _pack_x(query[b].T),
            "xk": _pack_x(key_[b].T),
            "xv": _pack_x(value[b].T),
            "wq": _pack_w(Wq[rows].T),
            "wk": _pack_w(Wk[rows].T),
            "wv": _pack_w(Wv[rows].T),
            "wo": np.ascontiguousarray(
                Wo[:, rows].T.reshape(HPC, D, E).transpose(1, 0, 2)),
            "bqk": np.stack([bq[rows][:P], bq[rows][P:],
                             bk[rows][:P], bk[rows][P:]], axis=1),
            "vones": np.ones((P, HPC), np.float32),
        })

    nc = _get_nc()
    LAST_RESULT = run_bass_kernel_spmd(nc, in_maps, core_ids=list(range(NCORES)))

    out = np.zeros((B, S, E), np.float32)
    for c in range(NCORES):
        out[c // GROUPS] += LAST_RESULT.results[c]["outT"].T
    out += bv @ Wo.T + bo
    return out


# revision 23
# speedup vs baseline: 1.1483x; 1.1483x over previous
"""Decomposed multi-head attention for trn2, 8 NeuronCores.

Sharding: core c = b*4 + g handles batch b (of 2) and head-group g (4 heads,
256 dims of E). Each core computes its heads' q/k/v projections, attention
(scores -> exp -> attn@v with a ones-column appended to V for the softmax
denominator), and a partial out-projection over its 256-dim column block of
Wo. The four partials per batch are summed on the host; bq/bk are added on
device, bv/bo fold into a constant row added on the host (bv @ Wo.T + bo).

All device matmuls run as out = lhsT.T @ rhs with operands packed on the
host into the exact SBUF layouts (so every DMA is contiguous per partition)
and typed float32r for single-pass PE throughput. Attention is software-
pipelined: attn@v trails scores/exp by one k-tile so the PE never stalls on
ScalarE and the HAM clock gate stays open.
"""

import ml_dtypes
import numpy as np
from contextlib import ExitStack

import concourse.bass as bass
import concourse.bacc as bacc
import concourse.mybir as mybir
import concourse.tile as tile
from concourse.bass_utils import run_bass_kernel_spmd

B, S, E, H, D = 2, 2048, 1024, 16, 64
P = 128
NCORES = 8
GROUPS = 4                # head-groups across cores (x batch = 8 cores)
HPC = H // GROUPS         # 4 heads per core
EG = HPC * D              # 256 dims of E per core
KE = E // P               # 8 k-tiles for the projections
NB = 4                    # 512-wide S blocks in the projection phase
SW = S // NB              # 512
ST = S // P               # 16 S-tiles for v
KT = S // P               # 16 k-tiles in attention
QW = 1024                 # q-block width in attention
QB = S // QW              # 2 q-blocks
F32 = mybir.dt.float32
F32R = mybir.dt.float32r
BF16 = mybir.dt.bfloat16

_NC = None
LAST_RESULT = None        # BassKernelResults of the most recent run (for test.py)


def _build_program():
    nc = bacc.Bacc("TRN2", target_bir_lowering=False, debug=False,
                   num_devices=NCORES)
    AF = mybir.ActivationFunctionType

    # Host-packed layouts: every DMA below is contiguous per partition.
    xq = nc.declare_dram_parameter("xq", [NB, P, KE, SW], F32R, isOutput=False)[:]
    xk = nc.declare_dram_parameter("xk", [NB, P, KE, SW], F32R, isOutput=False)[:]
    xv = nc.declare_dram_parameter("xv", [NB, P, KE, SW], F32R, isOutput=False)[:]
    wq = nc.declare_dram_parameter("wq", [P, KE, EG], F32R, isOutput=False)[:]
    wk = nc.declare_dram_parameter("wk", [P, KE, EG], F32R, isOutput=False)[:]
    wv = nc.declare_dram_parameter("wv", [P, KE, EG], F32R, isOutput=False)[:]
    wo = nc.declare_dram_parameter("wo", [D, HPC, E], F32R, isOutput=False)[:]
    bqk = nc.declare_dram_parameter("bqk", [P, 4], F32, isOutput=False)[:]
    vones = nc.declare_dram_parameter("vones", [P, HPC], BF16, isOutput=False)[:]
    outT = nc.declare_dram_parameter("outT", [E, S], F32, isOutput=True)[:]

    out_t = outT.rearrange("(m p) s -> p m s", p=P)

    with tile.TileContext(nc) as tc, ExitStack() as ctx:
        wpool = ctx.enter_context(tc.tile_pool(name="w", bufs=1))
        xpool = ctx.enter_context(tc.tile_pool(name="x", bufs=2))
        qkpool = ctx.enter_context(tc.tile_pool(name="qk", bufs=1))
        vpool = ctx.enter_context(tc.tile_pool(name="v", bufs=1))
        expool = ctx.enter_context(tc.tile_pool(name="ex", bufs=6))
        ohpool = ctx.enter_context(tc.tile_pool(name="oh", bufs=1))
        npool = ctx.enter_context(tc.tile_pool(name="n", bufs=2))
        outpool = ctx.enter_context(tc.tile_pool(name="ot", bufs=2))
        psum = ctx.enter_context(tc.tile_pool(name="ps", bufs=2, space="PSUM"))
        opsum = ctx.enter_context(tc.tile_pool(name="os", bufs=2, space="PSUM"))

        # --- resident weights ---
        wq_sb = wpool.tile([P, KE, EG], F32R, tag="wq")
        nc.sync.dma_start(wq_sb[:], wq)
        wk_sb = wpool.tile([P, KE, EG], F32R, tag="wk")
        nc.sync.dma_start(wk_sb[:], wk)
        wv_sb = wpool.tile([P, KE, EG], F32R, tag="wv")
        nc.sync.dma_start(wv_sb[:], wv)
        # out-proj weights laid out per head-sized k-tile of 64 partitions
        wo_sb = wpool.tile([D, HPC, E], F32R, tag="wo")
        nc.sync.dma_start(wo_sb[:], wo)
        bqk_sb = wpool.tile([P, 4], F32, tag="bqk")
        nc.sync.dma_start(bqk_sb[:], bqk)

        # --- phase 1: q/k/v projections, streamed in 512-wide S blocks ---
        qTt = [[None] * NB for _ in range(2)]   # [m][nb] -> [P, 512] (dims, S)
        kTt = [[None] * NB for _ in range(2)]
        v_sb = [None] * ST                      # [P, HPC, D+1] per S-tile
        for nb in range(NB):
            xq_blk = xpool.tile([P, KE, SW], F32R, tag="xblk",
                                name=f"xq_blk{nb}")
            nc.sync.dma_start(xq_blk[:], xq[nb])
            for m in range(2):
                ps = psum.tile([P, SW], F32, tag="sc", name=f"psq{nb}{m}")
                for k in range(KE):
                    nc.tensor.matmul(
                        ps[:], wq_sb[:, k, m * P:(m + 1) * P],
                        xq_blk[:, k, :],
                        start=(k == 0), stop=(k == KE - 1))
                t = qkpool.tile([P, SW], BF16, tag=f"qT{m}{nb}",
                                name=f"qT{m}{nb}")
                qTt[m][nb] = t
                nc.vector.tensor_scalar_add(t[:], ps[:], bqk_sb[:, m:m + 1])
            xk_blk = xpool.tile([P, KE, SW], F32R, tag="xblk",
                                name=f"xk_blk{nb}")
            nc.sync.dma_start(xk_blk[:], xk[nb])
            for m in range(2):
                ps = psum.tile([P, SW], F32, tag="sc", name=f"psk{nb}{m}")
                for k in range(KE):
                    nc.tensor.matmul(
                        ps[:], wk_sb[:, k, m * P:(m + 1) * P],
                        xk_blk[:, k, :],
                        start=(k == 0), stop=(k == KE - 1))
                t = qkpool.tile([P, SW], BF16, tag=f"kT{m}{nb}",
                                name=f"kT{m}{nb}")
                kTt[m][nb] = t
                nc.vector.tensor_scalar_add(t[:], ps[:], bqk_sb[:, 2 + m:3 + m])
            xv_blk = xpool.tile([P, KE, SW], F32R, tag="xblk",
                                name=f"xv_blk{nb}")
            nc.sync.dma_start(xv_blk[:], xv[nb])
            for st4 in range(SW // P):
                st = nb * (SW // P) + st4
                ps = psum.tile([P, EG], F32, tag="sc", name=f"psv{st}")
                for k in range(KE):
                    nc.tensor.matmul(
                        ps[:], xv_blk[:, k, st4 * P:(st4 + 1) * P],
                        wv_sb[:, k, :],
                        start=(k == 0), stop=(k == KE - 1))
                vt = vpool.tile([P, HPC, D + 1], BF16, tag=f"v{st}",
                                name=f"v{st}")
                v_sb[st] = vt
                nc.sync.dma_start(vt[:, :, D:D + 1], vones.unsqueeze(2))
                nc.vector.tensor_copy(
                    vt[:, :, 0:D], ps[:].rearrange("p (h d) -> p h d", h=HPC))

        # --- phase 2: attention + out-projection per q-block ---
        # Heads run in pairs (2p, 2p+1): partition offsets 0/64 of the same
        # m-half, so their K=64 score matmuls use different PE row groups
        # and overlap. attn@v trails by one k-tile (software pipeline), so
        # every PE instruction's inputs are ready when it issues and the PE
        # stays continuously busy (HAM stays at K=8/8).
        for qb in range(QB):
            oh = [None] * HPC                   # [64, QW] per head
            for pr in range(HPC // 2):
                m = pr
                ps_o = [opsum.tile([D + 1, QW], F32, tag="oacc",
                                   name=f"ps_o{qb}{pr}{i}")
                        for i in range(2)]
                exs = {}                       # k -> [exA, exB]

                def attnv(kv):
                    for hh in range(2):
                        h = 2 * pr + hh
                        for j in range(2):
                            nc.tensor.matmul(
                                ps_o[hh][:, j * SW:(j + 1) * SW],
                                v_sb[kv][:, h, :],
                                exs[kv][hh][:, j * SW:(j + 1) * SW],
                                start=(kv == 0), stop=(kv == KT - 1))
                    del exs[kv]

                for k in range(KT):
                    ps_s = [psum.tile([P, QW], F32, tag="sc",
                                      name=f"ps_s{qb}{pr}{k}{i}")
                            for i in range(2)]
                    for hh in range(2):
                        po = hh * D
                        for j in range(2):
                            nc.tensor.matmul(
                                ps_s[hh][:, j * SW:(j + 1) * SW],
                                kTt[m][k // 4][po:po + D,
                                               (k % 4) * P:(k % 4 + 1) * P],
                                qTt[m][qb * 2 + j][po:po + D, :],
                                start=True, stop=True)
                    exs[k] = []
                    for hh in range(2):
                        ex = expool.tile([P, QW], BF16, tag="ex",
                                         name=f"ex{qb}{pr}{k}{hh}")
                        nc.scalar.activation(ex[:], ps_s[hh][:], AF.Exp,
                                             scale=float(1.0 / np.sqrt(D)))
                        exs[k].append(ex)
                    # attn@v trails by 2 k-tiles and is emitted in pairs so
                    # the PE sees ~4.4us dense stretches of full-K matmuls
                    # (lets the HAM activity window re-open the clock gate).
                    if k >= 3 and k % 2 == 1:
                        attnv(k - 3)
                        attnv(k - 2)
                for kv in (KT - 2, KT - 1):
                    attnv(kv)
                for hh in range(2):
                    h = 2 * pr + hh
                    rec = npool.tile([1, QW], F32, tag="rec",
                                     name=f"rec{qb}{h}")
                    nc.vector.reciprocal(rec[:], ps_o[hh][D:D + 1, :])
                    rb = npool.tile([D, QW], F32, tag="rb", name=f"rb{qb}{h}")
                    nc.gpsimd.partition_broadcast(rb[:], rec[:])
                    oht = ohpool.tile([D, QW], F32R, tag=f"oh{h}{qb}",
                                      name=f"oh{h}{qb}")
                    oh[h] = oht
                    nc.vector.tensor_mul(oht[:], ps_o[hh][0:D, :], rb[:])
            for mo in range(E // P):
                for nbq in range(QW // SW):
                    pso = psum.tile([P, SW], F32, tag="sc",
                                    name=f"pso{qb}{mo}{nbq}")
                    for kt in range(HPC):
                        nc.tensor.matmul(
                            pso[:], wo_sb[:, kt, mo * P:(mo + 1) * P],
                            oh[kt][:, nbq * SW:(nbq + 1) * SW],
                            start=(kt == 0), stop=(kt == HPC - 1))
                    ot = outpool.tile([P, SW], F32, tag="ot",
                                      name=f"ot{qb}{mo}{nbq}")
                    nc.vector.tensor_copy(ot[:], pso[:])
                    nc.sync.dma_start(
                        out_t[:, mo, qb * QW + nbq * SW: qb * QW + (nbq + 1) * SW],
                        ot[:])
    nc.compile()
    return nc


def _get_nc():
    global _NC
    if _NC is None:
        _NC = _build_program()
    return _NC


def _pack_x(xT):
    """[E, S] f32 -> [NB, P, KE, SW] so each DMA block is contiguous 16KB
    per partition: out[nb, p, k, s] = xT[k*P + p, nb*SW + s]."""
    return np.ascontiguousarray(
        xT.reshape(KE, P, NB, SW).transpose(2, 1, 0, 3))


def _pack_w(wT):
    """[E, EG] f32 -> [P, KE, EG]: out[p, k, m] = wT[k*P + p, m]."""
    return np.ascontiguousarray(wT.reshape(KE, P, EG).transpose(1, 0, 2))


def kernel(query, key_, value, Wq, bq, Wk, bk, Wv, bv, Wo, bo):
    global LAST_RESULT
    query = np.asarray(query, np.float32)
    key_ = np.asarray(key_, np.float32)
    value = np.asarray(value, np.float32)
    Wq, Wk, Wv, Wo = (np.asarray(w, np.float32) for w in (Wq, Wk, Wv, Wo))
    bq, bk, bv, bo = (np.asarray(b_, np.float32) for b_ in (bq, bk, bv, bo))

    in_maps = []
    for c in range(NCORES):
        b, g = divmod(c, GROUPS)
        rows = slice(g * EG, (g + 1) * EG)
        in_maps.append({
            "xq": _pack_x(query[b].T),
            "xk": _pack_x(key_[b].T),
            "xv": _pack_x(value[b].T),
            "wq": _pack_w(Wq[rows].T),
            "wk": _pack_w(Wk[rows].T),
            "wv": _pack_w(Wv[rows].T),
            "wo": np.ascontiguousarray(
                Wo[:, rows].T.reshape(HPC, D, E).transpose(1, 0, 2)),
            "bqk": np.stack([bq[rows][:P], bq[rows][P:],
                             bk[rows][:P], bk[rows][P:]], axis=1),
            "vones": np.ones((P, HPC), ml_dtypes.bfloat16),
        })

    nc = _get_nc()
    LAST_RESULT = run_bass_kernel_spmd(nc, in_maps, core_ids=list(range(NCORES)))

    out = np.zeros((B, S, E), np.float32)
    for c in range(NCORES):
        out[c // GROUPS] += LAST_RESULT.results[c]["outT"].T
    out += bv @ Wo.T + bo
    return out
